# revision 1
# baseline (speedup 1.0000x reference)
"""Multi-head attention (B=2, N=2048, D=768, H=12, Dh=64) on 8 TRN2 NeuronCores.

Sharding: head-parallel Megatron-style. Core c handles batch b=c//4 and heads
[3*(c%4), 3*(c%4)+3). Each core projects q/k/v for its 3 heads (column-sliced
Wq/Wkv), runs softmax(q k^T/8) v on-chip, and computes a partial out-projection
against its row-slice of Wproj. Host sums the 4 partials per batch + bias.

On-chip layout: sources are host-pre-transposed so projections are natural
matmuls. Scores are computed transposed (S^T: k on partitions, q free) so the
attn@v matmul consumes exp(S^T) directly as the streaming operand with
lhsT = [v | ones]; the ones column yields the softmax denominator for free.
"""

import os
import sys

sys.path.insert(0, "/opt/trn_rl_repo")

from contextlib import ExitStack

import ml_dtypes
import numpy as np

import concourse.bass as bass
import concourse.bacc as bacc
import concourse.tile as tile
from concourse import mybir
from concourse.bass_utils import run_bass_kernel_spmd

bf16 = ml_dtypes.bfloat16
F32 = mybir.dt.float32
BF16 = mybir.dt.bfloat16
EXP = mybir.ActivationFunctionType.Exp

P = 128          # partitions
NQ = 2048        # query length (per batch)
NKV = 2048       # kv length
D = 768          # model dim
DH = 64          # head dim
HL = 3           # heads per core
DL = HL * DH     # local projected dim (192)
KB = D // P      # contraction blocks for projections (6)
NKB = NKV // P   # k-index blocks (16)
QC = 1024        # q chunk for the attention inner loop
NQC = NQ // QC   # 2
SCALE = DH ** -0.5

_CACHE: dict = {}
LAST_RESULTS = None


def _build_program() -> bass.Bass:
    nc = bacc.Bacc("TRN2", target_bir_lowering=False)

    qsT = nc.dram_tensor("qsT", [D, NQ], BF16, kind="ExternalInput")
    kvT = nc.dram_tensor("kvT", [D, NKV], BF16, kind="ExternalInput")
    wq = nc.dram_tensor("wq", [D, DL], BF16, kind="ExternalInput")
    wk = nc.dram_tensor("wk", [D, DL], BF16, kind="ExternalInput")
    wv = nc.dram_tensor("wv", [D, DL], BF16, kind="ExternalInput")
    wp = nc.dram_tensor("wp", [DL, D], BF16, kind="ExternalInput")
    out = nc.dram_tensor("out", [NQ, D], BF16, kind="ExternalOutput")

    with tile.TileContext(nc) as tc, ExitStack() as ctx:
        sb_src = ctx.enter_context(tc.tile_pool(name="src", bufs=KB))
        sb_w = ctx.enter_context(tc.tile_pool(name="wts", bufs=KB))
        sb_p = ctx.enter_context(tc.tile_pool(name="persist", bufs=1))
        sb_es = ctx.enter_context(tc.tile_pool(name="es", bufs=6))
        sb_sm = ctx.enter_context(tc.tile_pool(name="small", bufs=3))
        sb_ob = ctx.enter_context(tc.tile_pool(name="outsb", bufs=4))

        # ---- DMA inputs ----
        # Spread input DMAs across engine DGE queues; whole-tensor strided
        # DMAs for the small weights. Priority: q-half-0 columns first.
        qsT_sb, kvT_sb = [], []
        for kb in range(KB):
            t_kv = sb_src.tile([P, NKV], BF16, tag="kvT")
            kvT_sb.append(t_kv)
            t_qs = sb_src.tile([P, NQ], BF16, tag="qsT")
            qsT_sb.append(t_qs)
        wkt = sb_w.tile([P, KB * DL], BF16, tag="wk")
        wvt = sb_w.tile([P, KB * DL], BF16, tag="wv")
        wqt = sb_w.tile([P, KB * DL], BF16, tag="wq")
        for t, dram in ((wkt, wk), (wvt, wv), (wqt, wq)):
            nc.scalar.dma_start(
                t[:].rearrange("p (k c) -> p k c", k=KB),
                dram[:].rearrange("(k p) c -> p k c", k=KB),
            )
        wk_sb = [wkt[:, kb * DL : (kb + 1) * DL] for kb in range(KB)]
        wv_sb = [wvt[:, kb * DL : (kb + 1) * DL] for kb in range(KB)]
        wq_sb = [wqt[:, kb * DL : (kb + 1) * DL] for kb in range(KB)]
        qs = [nc.sync, nc.gpsimd, nc.scalar]
        for kb in range(KB):
            qs[kb % 3].dma_start(kvT_sb[kb][:, 0:QC], kvT[kb * P : (kb + 1) * P, 0:QC])
        for kb in range(KB):
            qs[kb % 3].dma_start(qsT_sb[kb][:, 0:QC], qsT[kb * P : (kb + 1) * P, 0:QC])
        for kb in range(KB):
            qs[kb % 3].dma_start(kvT_sb[kb][:, QC:NKV], kvT[kb * P : (kb + 1) * P, QC:NKV])
        for kb in range(KB):
            qs[kb % 3].dma_start(qsT_sb[kb][:, QC:NQ], qsT[kb * P : (kb + 1) * P, QC:NQ])
        wp01 = sb_p.tile([P, D], BF16, tag="wp01")
        nc.scalar.dma_start(wp01[:], wp[0:P, :])
        wp2 = sb_p.tile([DH, D], BF16, tag="wp2")
        nc.scalar.dma_start(wp2[:], wp[P : P + DH, :])

        # ---- persistent intermediates ----
        qT01 = sb_p.tile([P, NQ], BF16, tag="qT01")   # q^T heads 0,1 (d on partitions)
        kT01 = sb_p.tile([P, NKV], BF16, tag="kT01")
        qT2 = sb_p.tile([DH, NQ], BF16, tag="qT2")    # q^T head 2
        kT2 = sb_p.tile([DH, NKV], BF16, tag="kT2")
        vA = sb_p.tile([P, HL * NKB * 65], BF16, tag="vA")  # per (h, kb): [v(64) | ones]
        X01 = sb_p.tile([P, NQ], BF16, tag="X01")     # normalized x^T heads 0,1
        X2 = sb_p.tile([DH, NQ], BF16, tag="X2")
        nc.vector.memset(vA[:], 1.0)  # ones columns; v evacs overwrite the rest

        # Two kernel-lifetime PSUM pools (no phase barriers):
        #   psA (2 slots x 2 banks): scores + kT01/qT01-h0 startup chains
        #   psB (2 slots x 2 banks): v-proj, attn@v accumulators, dripped fillers
        psA = ctx.enter_context(tc.tile_pool(name="psA", bufs=2, space="PSUM"))
        psB = ctx.enter_context(tc.tile_pool(name="psB", bufs=2, space="PSUM"))

        def proj01_chain(pool, w_sb, src_sb, dst, half, evac):
            """(128,1024) chain: dst[:, half] = (w block cols 0:128).T @ srcT."""
            ps = pool.tile([P, QC], F32, tag="A" if pool is psA else "B")
            for kb in range(KB):
                for j in range(QC // 512):
                    nc.tensor.matmul(
                        ps[:, j * 512 : (j + 1) * 512],
                        w_sb[kb][:, 0:P],
                        src_sb[kb][:, half * QC + j * 512 : half * QC + (j + 1) * 512],
                        start=(kb == 0),
                        stop=(kb == KB - 1),
                    )
            evac(dst[:, half * QC : (half + 1) * QC], ps[:])

        def proj2_chain(pool, half):
            """col-tiled pair: qT2 (psum 0:64) / kT2 (psum 64:128) for one q-half."""
            ps = pool.tile([P, QC], F32, tag="A" if pool is psA else "B")
            for kb in range(KB):
                for j in range(QC // 512):
                    sl = slice(j * 512, (j + 1) * 512)
                    src_sl = slice(half * QC + j * 512, half * QC + (j + 1) * 512)
                    nc.tensor.matmul(
                        ps[0:DH, sl], wq_sb[kb][:, P:DL], qsT_sb[kb][:, src_sl],
                        start=(kb == 0), stop=(kb == KB - 1),
                    )
                    nc.tensor.matmul(
                        ps[DH:P, sl], wk_sb[kb][:, P:DL], kvT_sb[kb][:, src_sl],
                        start=(kb == 0), stop=(kb == KB - 1),
                    )
            nc.vector.tensor_copy(qT2[:, half * QC : (half + 1) * QC], ps[0:DH, :])
            nc.vector.tensor_copy(kT2[:, half * QC : (half + 1) * QC], ps[DH:P, :])

        vA_view = vA[:].rearrange("p (h k c) -> p h k c", h=HL, k=NKB)

        def v_chain(m):
            ps = psB.tile([P, DL], F32, tag="B")
            for kb in range(KB):
                nc.tensor.matmul(
                    ps[:], kvT_sb[kb][:, m * P : (m + 1) * P], wv_sb[kb][:],
                    start=(kb == 0), stop=(kb == KB - 1),
                )
            nc.vector.tensor_copy(
                vA_view[:, :, m, 0:DH],
                ps[:].rearrange("p (h d) -> p h d", h=HL),
            )

        def outproj_tile(m, pool, evac=None):
            po = pool.tile([P, D], F32, tag="A" if pool is psA else "B")
            for j, n in ((0, 512), (512, 256)):
                nc.tensor.matmul(
                    po[:, j : j + n], X01[:, m * P : (m + 1) * P], wp01[:, j : j + n],
                    start=True, stop=False,
                )
                nc.tensor.matmul(
                    po[:, j : j + n], X2[:, m * P : (m + 1) * P], wp2[:, j : j + n],
                    start=False, stop=True,
                )
            ob = sb_ob.tile([P, D], BF16, tag="ob")
            (evac or nc.vector.tensor_copy)(ob[:], po[:])
            nc.sync.dma_start(out[m * P : (m + 1) * P, :], ob[:])

        # ---- startup: the minimum needed for head-0 / q-half-0 scores ----
        proj01_chain(psA, wk_sb, kvT_sb, kT01, 0, nc.vector.tensor_copy)   # scores kb 0..7
        proj01_chain(psA, wq_sb, qsT_sb, qT01, 0, nc.vector.tensor_copy)   # q-half 0
        v_chain(0)

        # Fillers dripped into the attention kb-loops (emitted at given kb index).
        f00 = {kb: [(lambda m=kb + 1: v_chain(m))] for kb in range(0, 15)}
        f00[3] = f00[3] + [lambda: proj01_chain(psB, wk_sb, kvT_sb, kT01, 1, nc.scalar.copy)]
        fillers = {
            (0, 0): f00,
            (0, 1): {
                4: [lambda: proj01_chain(psB, wq_sb, qsT_sb, qT01, 1, nc.vector.tensor_copy)],
                9: [lambda: proj2_chain(psB, 0)],
            },
            (0, 2): {2: [lambda: proj2_chain(psB, 1)]},
            (1, 0): {kb: [(lambda m=kb // 2 - 2: outproj_tile(m, psB))] for kb in (4, 6, 8, 10)},
            (1, 1): {kb: [(lambda m=2 + kb // 2: outproj_tile(m, psB))] for kb in (4, 6, 8, 10)},
        }

        for qc in range(NQC):
            for h in range(HL):
                if h < 2:
                    kT_h = kT01[h * DH : (h + 1) * DH, :]
                    qT_h = qT01[h * DH : (h + 1) * DH, :]
                    X_h = X01[h * DH : (h + 1) * DH, :]
                else:
                    kT_h, qT_h, X_h = kT2[:], qT2[:], X2[:]
                drip = fillers.get((qc, h), {})
                xps = psB.tile([65, QC], F32, tag="B")
                for kb in range(NKB):
                    sc = psA.tile([P, QC], F32, tag="A")
                    for j in range(QC // 512):
                        nc.tensor.matmul(
                            sc[:, j * 512 : (j + 1) * 512],
                            kT_h[:, kb * P : (kb + 1) * P],
                            qT_h[:, qc * QC + j * 512 : qc * QC + (j + 1) * 512],
                            start=True, stop=True,
                        )
                    for fn in drip.get(kb, ()):
                        fn()
                    es = sb_es.tile([P, QC], BF16, tag="es")
                    nc.scalar.activation(es[:], sc[:], EXP, scale=SCALE)
                    for j in range(QC // 512):
                        sl = slice(j * 512, (j + 1) * 512)
                        nc.tensor.matmul(
                            xps[:, sl],
                            vA[:, (h * NKB + kb) * 65 : (h * NKB + kb + 1) * 65],
                            es[:, sl],
                            start=(kb == 0), stop=(kb == NKB - 1),
                        )
                # softmax denominator -> reciprocal -> broadcast -> normalize.
                # For the very last head, run the chain per 512-col chunk so the
                # final out-proj tiles can start after chunk 0 instead of the
                # whole-width chain.
                chunks = 2 if (qc, h) == (NQC - 1, HL - 1) else 1
                w = QC // chunks
                for j in range(chunks):
                    sl = slice(j * w, (j + 1) * w)
                    dn = sb_sm.tile([1, QC], F32, tag="dn")
                    nc.vector.tensor_copy(dn[0:1, 0:w], xps[64:65, sl])
                    rc = sb_sm.tile([1, QC], F32, tag="rc")
                    nc.vector.reciprocal_approx_fast(rc[0:1, 0:w], dn[0:1, 0:w])
                    bcs = sb_sm.tile([DH, QC], F32, tag="bcs")
                    nc.gpsimd.partition_broadcast(bcs[0:DH, 0:w], rc[0:1, 0:w])
                    nc.vector.tensor_mul(
                        X_h[:, qc * QC + j * w : qc * QC + (j + 1) * w],
                        xps[0:DH, sl], bcs[0:DH, 0:w],
                    )
        # remaining out-projection tiles (alternate evac engines at the tail)
        for m in range(8, NKB):
            outproj_tile(m, psA, evac=(nc.scalar.copy if m % 2 else None))

    nc.compile()
    return nc


def _get_nc() -> bass.Bass:
    if "nc" not in _CACHE:
        _CACHE["nc"] = _build_program()
    return _CACHE["nc"]


def kernel(**inputs) -> np.ndarray:
    global LAST_RESULTS
    qs = np.asarray(inputs["query_source"], dtype=np.float32)
    kv = np.asarray(inputs["kv_source"], dtype=np.float32)
    Wq = np.asarray(inputs["Wq"], dtype=np.float32)
    Wkv = np.asarray(inputs["Wkv"], dtype=np.float32)
    Wp = np.asarray(inputs["Wproj"], dtype=np.float32)
    bp = np.asarray(inputs["bproj"], dtype=np.float32)

    nc = _get_nc()
    in_maps = []
    for c in range(8):
        b = c // 4
        c0 = (c % 4) * DL
        in_maps.append(
            {
                "qsT": np.ascontiguousarray(qs[b].T).astype(bf16),
                "kvT": np.ascontiguousarray(kv[b].T).astype(bf16),
                "wq": Wq[:, c0 : c0 + DL].astype(bf16),
                "wk": Wkv[:, c0 : c0 + DL].astype(bf16),
                "wv": Wkv[:, D + c0 : D + c0 + DL].astype(bf16),
                "wp": Wp[c0 : c0 + DL, :].astype(bf16),
            }
        )

    trace = bool(int(os.environ.get("KERNEL_TRACE", "0")))
    res = run_bass_kernel_spmd(nc, in_maps, list(range(8)), trace=trace)
    LAST_RESULTS = res

    out = np.tile(bp.astype(np.float32), (2, NQ, 1))
    for c in range(8):
        out[c // 4] += res.results[c]["out"].astype(np.float32)
    return out



# revision 16
# speedup vs baseline: 1.1626x; 1.1626x over previous
"""Multi-head attention (B=2, N=2048, D=768, H=12, Dh=64) on 8 TRN2 NeuronCores.

Sharding: head-parallel Megatron-style. Core c handles batch b=c//4 and heads
[3*(c%4), 3*(c%4)+3). Each core projects q/k/v for its 3 heads (column-sliced
Wq/Wkv), runs softmax(q k^T/8) v on-chip, and computes a partial out-projection
against its row-slice of Wproj. Host sums the 4 partials per batch + bias.

fp8 attention core:
- q/k are projected in bf16 then evacuated to fp8(e4m3) in [dh, tok] layout.
  Scores run as fp8 DoubleRow matmuls with a zeroed second weight slot
  (contraction dh=64 in slot 0, slot 1 multiplies by zeros): 0.5 cycles/row,
  2x bf16.
- exp(S/8) lands in fp8 "es" tiles; kv-block PAIRS share one es tile
  [128, 2048] (slot 0 = even block, slot 1 = odd block) so attn@v is a real
  fp8 DoubleRow contraction of 256 kv tokens per instruction (4x bf16). The
  stationary operand is [v_even|ones(64)|v_odd|ones(64)] per head/pair; the
  64-wide ones block accumulates the softmax denominator REPLICATED across
  psum partitions 64:128, so normalization is reciprocal+multiply on DVE
  with no partition broadcast.
- exp is split across engines: Activation (table exp) plus DVE/Pool running
  a Schraudolph bit-trick exp (affine + round to int8, bitcast e4m3),
  accurate to ~the e4m3 quantization step.
- attn@v is software-pipelined one kv-pair behind scores so its blocked
  matmuls don't clog the PE wait queue; projection chains are dripped into
  the loop in 2-3 kb pieces (at most one chain in flight per head) to keep
  psum slots rotating for the exp pipeline.
- inputs are staged with 6 large DMAs (weights packed host-side) because
  each DMA trigger costs ~0.6us of serialized descriptor generation.
"""

import os
import sys

sys.path.insert(0, "/opt/trn_rl_repo")

from contextlib import ExitStack

import ml_dtypes
import numpy as np

import concourse.bass as bass
import concourse.bacc as bacc
import concourse.tile as tile
from concourse import mybir
from concourse.bass_utils import run_bass_kernel_spmd

bf16 = ml_dtypes.bfloat16
F32 = mybir.dt.float32
BF16 = mybir.dt.bfloat16
F8 = mybir.dt.float8e4
I8 = mybir.dt.int8
EXP = mybir.ActivationFunctionType.Exp
DR = mybir.MatmulPerfMode.DoubleRow
MUL = mybir.AluOpType.mult
ADD = mybir.AluOpType.add

P = 128          # partitions
NQ = 2048        # query length (per batch)
NKV = 2048       # kv length
D = 768          # model dim
DH = 64          # head dim
HL = 3           # heads per core
DL = HL * DH     # local projected dim (192)
KB = D // P      # contraction blocks for projections (6)
NKB = NKV // P   # k-index blocks (16)
NM2 = NKB // 2   # kv-block pairs (8)
QC = 1024        # q chunk for the attention inner loop
NQC = NQ // QC   # 2
QH = QC + 512    # qT8 per-half stride (data + zero pad; also all-zero DR rhs)
WQKV = 3 * DL    # packed weight row width (576)
SCALE = DH ** -0.5

# Schraudolph constants for exp(x*SCALE) -> e4m3 bits (bias 7, 3-bit mantissa)
A8 = float(SCALE * 8.0 / np.log(2.0))
B8 = float(7.0 * 8.0 - 0.0437695 * 8.0)

# exp engine split: kb -> engine (default Activation). GPSIMD cannot read
# PSUM, so only ACT and DVE can run the exp; odd kbs go to DVE so each
# kv-pair's two exps run concurrently on different engines.
EXP_DVE_KB = (1, 3, 5, 9, 11, 13)
EXP_DVE_KB_LAST = (1, 3, 5, 9, 11)

_CACHE: dict = {}
LAST_RESULTS = None


def _build_program() -> bass.Bass:
    nc = bacc.Bacc("TRN2", target_bir_lowering=False)

    qsT = nc.dram_tensor("qsT", [D, NQ], BF16, kind="ExternalInput")
    kvT = nc.dram_tensor("kvT", [D, NKV], BF16, kind="ExternalInput")
    wqkv = nc.dram_tensor("wqkv", [D, WQKV], BF16, kind="ExternalInput")
    wpp = nc.dram_tensor("wpp", [P, 2 * D], BF16, kind="ExternalInput")
    out = nc.dram_tensor("out", [NQ, D], BF16, kind="ExternalOutput")

    with tile.TileContext(nc) as tc, ExitStack() as ctx:
        sb_src = ctx.enter_context(tc.tile_pool(name="src", bufs=1))
        sb_p = ctx.enter_context(tc.tile_pool(name="persist", bufs=1))
        sb_es = ctx.enter_context(tc.tile_pool(name="es", bufs=4))
        sb_sm = ctx.enter_context(tc.tile_pool(name="small", bufs=3))
        sb_ob = ctx.enter_context(tc.tile_pool(name="outsb", bufs=4))

        # ---- DMA inputs: few big transfers (trigger desc-gen is ~0.6us each)
        kvT_sb = sb_src.tile([P, KB * NKV], BF16, tag="kvT")
        qsT_sb = sb_src.tile([P, KB * NQ], BF16, tag="qsT")
        wqkv_sb = sb_src.tile([P, KB * WQKV], BF16, tag="wqkv")
        wpx = sb_src.tile([P, 2 * D], BF16, tag="wpx")

        kv_v = kvT_sb[:].rearrange("p (k c) -> p k c", k=KB)
        qs_v = qsT_sb[:].rearrange("p (k c) -> p k c", k=KB)
        kv_d = kvT[:].rearrange("(k p) c -> p k c", k=KB)
        qs_d = qsT[:].rearrange("(k p) c -> p k c", k=KB)
        nc.scalar.dma_start(
            wqkv_sb[:].rearrange("p (k c) -> p k c", k=KB),
            wqkv[:].rearrange("(k p) c -> p k c", k=KB),
        )
        nc.sync.dma_start(kv_v[:, :, 0:QC], kv_d[:, :, 0:QC])
        nc.scalar.dma_start(qs_v[:, :, 0:QC], qs_d[:, :, 0:QC])
        nc.sync.dma_start(kv_v[:, :, QC:NKV], kv_d[:, :, QC:NKV])
        nc.scalar.dma_start(wpx[:], wpp[:])
        nc.scalar.dma_start(qs_v[:, :, QC:NQ], qs_d[:, :, QC:NQ])

        def w_sl(which, kb):  # 0=q, 1=k, 2=v
            base = kb * WQKV + which * DL
            return wqkv_sb[:, base : base + DL]

        wp01 = wpx[:, 0:D]
        wp2 = wpx[0:DH, D : 2 * D]

        # ---- persistent fp8 intermediates ----
        kT8 = sb_p.tile([P, NKV + P], F8, tag="kT8")       # heads 0,1; zero tail
        kT8_2 = sb_p.tile([DH, NKV + P], F8, tag="kT8_2")  # head 2
        qT8 = sb_p.tile([P, NQC * QH], F8, tag="qT8")      # halves at 0, QH
        qT8_2 = sb_p.tile([DH, NQC * QH], F8, tag="qT8_2")
        # per (h, pair): [v_even(64) | ones(64) | v_odd(64) | ones(64)]
        vA8 = sb_p.tile([P, HL * NM2 * 256], F8, tag="vA8")
        X01 = sb_p.tile([P, NQ], BF16, tag="X01")
        X2 = sb_p.tile([DH, NQ], BF16, tag="X2")

        # zero tails (read by score matmul slot-1 APs; contribution is x0)
        nc.gpsimd.memset(kT8[:, NKV : NKV + P], 0.0)
        nc.gpsimd.memset(kT8_2[:, NKV : NKV + P], 0.0)
        for half in range(NQC):
            nc.gpsimd.memset(qT8[:, half * QH + QC : (half + 1) * QH], 0.0)
            nc.gpsimd.memset(qT8_2[:, half * QH + QC : (half + 1) * QH], 0.0)
        # ones blocks of vA8 (softmax denominator, replicated across 64 rows)
        ones_v = vA8[:].rearrange("p (s c) -> p s c", c=P)[:, :, DH:P]
        nc.gpsimd.memset(ones_v[:], 1.0)

        psA = ctx.enter_context(tc.tile_pool(name="psA", bufs=4, space="PSUM"))

        def proj01_chain(w_which, dst8, half):
            """(128,1024) startup chain: dst8 half = (w cols 0:128).T @ srcT."""
            src_sb = kvT_sb if dst8 is kT8 else qsT_sb
            src_w = NKV
            ps = psA.tile([P, QC], F32, tag="A", name="ps")
            for kb in range(KB):
                for j in range(QC // 512):
                    nc.tensor.matmul(
                        ps[:, j * 512 : (j + 1) * 512],
                        w_sl(w_which, kb)[:, 0:P],
                        src_sb[:, kb * src_w + half * QC + j * 512 : kb * src_w + half * QC + (j + 1) * 512],
                        start=(kb == 0),
                        stop=(kb == KB - 1),
                    )
            dst_off = half * (QH if dst8 is qT8 else QC)
            nc.vector.tensor_copy(dst8[:, dst_off : dst_off + QC], ps[:])

        # piecewise chains: a few kb blocks per drip point; psum tile held
        # across pieces via chain_state.
        chain_state = {}

        def proj01_piece(key, w_which, dst8, half, kbs):
            if key not in chain_state:
                chain_state[key] = psA.tile([P, QC], F32, tag="A", name="ps")
            ps = chain_state[key]
            src_sb = kvT_sb if dst8 is kT8 else qsT_sb
            for kb in kbs:
                for j in range(QC // 512):
                    nc.tensor.matmul(
                        ps[:, j * 512 : (j + 1) * 512],
                        w_sl(w_which, kb)[:, 0:P],
                        src_sb[:, kb * NKV + half * QC + j * 512 : kb * NKV + half * QC + (j + 1) * 512],
                        start=(kb == 0),
                        stop=(kb == KB - 1),
                    )
            if kbs[-1] == KB - 1:
                dst_off = half * (QH if dst8 is qT8 else QC)
                nc.vector.tensor_copy(dst8[:, dst_off : dst_off + QC], ps[:])
                del chain_state[key]

        def proj2_piece(key, half, kbs):
            if key not in chain_state:
                chain_state[key] = psA.tile([P, QC], F32, tag="A", name="ps")
            ps = chain_state[key]
            for kb in kbs:
                for j in range(QC // 512):
                    sl = slice(j * 512, (j + 1) * 512)
                    qsl = slice(kb * NQ + half * QC + j * 512, kb * NQ + half * QC + (j + 1) * 512)
                    nc.tensor.matmul(
                        ps[0:DH, sl], w_sl(0, kb)[:, P:DL], qsT_sb[:, qsl],
                        start=(kb == 0), stop=(kb == KB - 1),
                    )
                    nc.tensor.matmul(
                        ps[DH:P, sl], w_sl(1, kb)[:, P:DL], kvT_sb[:, qsl],
                        start=(kb == 0), stop=(kb == KB - 1),
                    )
            if kbs[-1] == KB - 1:
                nc.vector.tensor_copy(qT8_2[:, half * QH : half * QH + QC], ps[0:DH, :])
                nc.vector.tensor_copy(kT8_2[:, half * QC : half * QC + QC], ps[DH:P, :])
                del chain_state[key]

        vA8_hview = vA8[:].rearrange("p (h r) -> p h r", h=HL)

        def v_chain(m):
            ps = psA.tile([P, DL], F32, tag="A", name="ps")
            for kb in range(KB):
                nc.tensor.matmul(
                    ps[:], kvT_sb[:, kb * NKV + m * P : kb * NKV + (m + 1) * P],
                    w_sl(2, kb),
                    start=(kb == 0), stop=(kb == KB - 1),
                )
            off = (m // 2) * 256 + (m % 2) * P
            nc.vector.tensor_copy(
                vA8_hview[:, :, off : off + DH],
                ps[:].rearrange("p (h d) -> p h d", h=HL),
            )

        def outproj_tile(m):
            po = psA.tile([P, D], F32, tag="A", name="po")
            for j, n in ((0, 512), (512, 256)):
                nc.tensor.matmul(
                    po[:, j : j + n], X01[:, m * P : (m + 1) * P], wp01[:, j : j + n],
                    start=True, stop=False,
                )
                nc.tensor.matmul(
                    po[:, j : j + n], X2[:, m * P : (m + 1) * P], wp2[:, j : j + n],
                    start=False, stop=True,
                )
            ob = sb_ob.tile([P, D], BF16, tag="ob")
            (nc.vector.tensor_copy if m % 2 else nc.scalar.copy)(ob[:], po[:])
            nc.gpsimd.dma_start(out[m * P : (m + 1) * P, :], ob[:])

        # ---- startup ----
        proj01_chain(1, kT8, 0)
        proj01_chain(0, qT8, 0)
        v_chain(0)
        v_chain(1)

        # fillers keyed (qc, h) -> {m2: [fns]}; at most one chain per head so
        # the psum pool keeps >=2 slots rotating for scores.
        f00 = {m2: [(lambda a=2 * m2 + 2: v_chain(a)), (lambda a=2 * m2 + 3: v_chain(a))]
               for m2 in range(0, 7)}
        for m2, kbs in ((0, (0, 1, 2)), (1, (3, 4, 5))):
            f00[m2] = f00[m2] + [lambda kbs=kbs: proj01_piece("k1", 1, kT8, 1, kbs)]
        fillers = {
            (0, 0): f00,
            (0, 1): {
                0: [lambda: proj2_piece("p20", 0, (0, 1))],
                1: [lambda: proj2_piece("p20", 0, (2, 3))],
                2: [lambda: proj2_piece("p20", 0, (4, 5))],
                4: [lambda: proj2_piece("p21", 1, (0, 1))],
                5: [lambda: proj2_piece("p21", 1, (2, 3))],
                6: [lambda: proj2_piece("p21", 1, (4, 5))],
            },
            (0, 2): {
                0: [lambda: proj01_piece("q1", 0, qT8, 1, (0, 1, 2))],
                1: [lambda: proj01_piece("q1", 0, qT8, 1, (3, 4, 5))],
            },
            (1, 0): {m2: [(lambda m=m2 - 2: outproj_tile(m))] for m2 in (2, 3, 4, 5)},
            (1, 1): {m2: [(lambda m=m2 + 2: outproj_tile(m))] for m2 in (2, 3, 4, 5)},
        }

        def k_lhsT(kT8_t, h, kb):
            """[64, 2, 128] fp8: slot0 = k block kb, slot1 = zero tail at NKV."""
            rows = kT8_t[h * DH : (h + 1) * DH, :] if kT8_t is kT8 else kT8_t[:]
            nstep = NKB - kb  # slot step in 128-col units
            v = rows[:, kb * P : kb * P + (nstep + 1) * P]
            v = v.rearrange("p (s c) -> p s c", c=P)
            return v[:, 0 :: nstep, :] if nstep > 1 else v

        def q_rhs(qT8_t, h, qc, j):
            """[64, 2, 256] fp8: slot0 = q cols, slot1 = next 256 (zero-weighted)."""
            rows = qT8_t[h * DH : (h + 1) * DH, :] if qT8_t is qT8 else qT8_t[:]
            off = qc * QH + j * 256
            return rows[:, off : off + 512].rearrange("p (s c) -> p s c", s=2)

        for qc in range(NQC):
            for h in range(HL):
                kT_t = kT8 if h < 2 else kT8_2
                qT_t = qT8 if h < 2 else qT8_2
                X_h = X01[h * DH : (h + 1) * DH, :] if h < 2 else X2[:]
                drip = fillers.get((qc, h), {})
                xps = psA.tile([P, QC], F32, tag="A")

                def va_of(m2):
                    va = vA8[:, (h * NM2 + m2) * 256 : (h * NM2 + m2 + 1) * 256]
                    return va.rearrange("p (s c) -> p s c", s=2)

                # [128, 2, 256] all-zero rhs (qT8 pad): a matmul against it
                # contributes 0 and doubles as a weight-load warmup -- the
                # first DR matmul after a wide dual-plane ldweights can stream
                # before the load settles, so never let a live attn@v matmul
                # be first after a vA8 weight switch.
                zrhs = qT8[:, QC : QC + 512].rearrange("p (s c) -> p s c", s=2)

                # explicit zeroing of the xps accumulator (all attn@v matmuls
                # then accumulate with start=False, in any order)
                for j in range(QC // 256):
                    nc.tensor.matmul(
                        xps[:, j * 256 : (j + 1) * 256], va_of(0), zrhs,
                        start=True, stop=False, perf_mode=DR, skip_group_check=True,
                    )

                def attnv(m2, es2, js):
                    va = va_of(m2)
                    nc.tensor.matmul(  # warmup: +0, loads weights
                        xps[:, 0:256], va, zrhs,
                        start=False, stop=False, perf_mode=DR, skip_group_check=True,
                    )
                    es_v = es2[:].rearrange("p (s c) -> p s c", s=2)
                    for j in js:
                        nc.tensor.matmul(
                            xps[:, j * 256 : (j + 1) * 256],
                            va,
                            es_v[:, :, j * 256 : (j + 1) * 256],
                            start=False, stop=(m2 == NM2 - 1), perf_mode=DR,
                            skip_group_check=True,
                        )

                pend = None  # software pipeline: attn@v trails scores one pair
                for m2 in range(NM2):
                    es2 = sb_es.tile([P, 2 * QC], F8, tag="es")
                    for half in range(2):
                        kb = 2 * m2 + half
                        sc = psA.tile([P, QC], F32, tag="A")
                        for j in range(QC // 256):
                            nc.tensor.matmul(
                                sc[:, j * 256 : (j + 1) * 256],
                                k_lhsT(kT_t, h, kb),
                                q_rhs(qT_t, h, qc, j),
                                start=True, stop=True, perf_mode=DR,
                            )
                        if pend is not None:
                            attnv(pend[0], pend[1], (0, 1) if half == 0 else (2, 3))
                        dst = es2[:, half * QC : (half + 1) * QC]
                        last = (qc, h) == (NQC - 1, HL - 1)
                        dve_kb = EXP_DVE_KB_LAST if last else EXP_DVE_KB
                        if kb in dve_kb:
                            nc.vector.tensor_scalar(dst.bitcast(I8), sc[:], A8, B8, MUL, ADD)
                        else:
                            nc.scalar.activation(dst, sc[:], EXP, scale=SCALE)
                    for fn in drip.get(m2, ()):
                        fn()
                    pend = (m2, es2)
                attnv(pend[0], pend[1], (0, 1, 2, 3))

                # denominator (replicated in xps rows 64:128) -> reciprocal ->
                # normalize, all on DVE
                chunks = 2 if (qc, h) == (NQC - 1, HL - 1) else 1
                w = QC // chunks
                for j in range(chunks):
                    sl = slice(j * w, (j + 1) * w)
                    dn = sb_sm.tile([DH, QC], F32, tag="dn")
                    nc.vector.tensor_copy(dn[0:DH, 0:w], xps[DH:P, sl])
                    rc = sb_sm.tile([DH, QC], F32, tag="rc")
                    nc.vector.reciprocal_approx_fast(rc[0:DH, 0:w], dn[0:DH, 0:w])
                    nc.vector.tensor_tensor(
                        X_h[:, qc * QC + j * w : qc * QC + (j + 1) * w],
                        xps[0:DH, sl], rc[0:DH, 0:w], MUL,
                    )
        # remaining out-projection tiles
        for m in range(8, NKB):
            outproj_tile(m)

    nc.compile()
    return nc


def _get_nc() -> bass.Bass:
    if "nc" not in _CACHE:
        _CACHE["nc"] = _build_program()
    return _CACHE["nc"]


def kernel(**inputs) -> np.ndarray:
    global LAST_RESULTS
    qs = np.asarray(inputs["query_source"], dtype=np.float32)
    kv = np.asarray(inputs["kv_source"], dtype=np.float32)
    Wq = np.asarray(inputs["Wq"], dtype=np.float32)
    Wkv = np.asarray(inputs["Wkv"], dtype=np.float32)
    Wp = np.asarray(inputs["Wproj"], dtype=np.float32)
    bp = np.asarray(inputs["bproj"], dtype=np.float32)

    nc = _get_nc()
    in_maps = []
    for c in range(8):
        b = c // 4
        c0 = (c % 4) * DL
        wqkv = np.concatenate(
            [Wq[:, c0 : c0 + DL], Wkv[:, c0 : c0 + DL], Wkv[:, D + c0 : D + c0 + DL]],
            axis=1,
        )
        wp_loc = Wp[c0 : c0 + DL, :]
        wpp = np.zeros((P, 2 * D), dtype=np.float32)
        wpp[:, 0:D] = wp_loc[0:P]
        wpp[0:DH, D : 2 * D] = wp_loc[P:DL]
        in_maps.append(
            {
                "qsT": np.ascontiguousarray(qs[b].T).astype(bf16),
                "kvT": np.ascontiguousarray(kv[b].T).astype(bf16),
                "wqkv": wqkv.astype(bf16),
                "wpp": wpp.astype(bf16),
            }
        )

    trace = bool(int(os.environ.get("KERNEL_TRACE", "0")))
    res = run_bass_kernel_spmd(nc, in_maps, list(range(8)), trace=trace)
    LAST_RESULTS = res

    out = np.tile(bp.astype(np.float32), (2, NQ, 1))
    for c in range(8):
        out[c // 4] += res.results[c]["out"].astype(np.float32)
    return out
